# revision 9
# baseline (speedup 1.0000x reference)
# Trainium2 Bass kernel for nn_CovariantPotentialNet (B=4096, D=64, K=64, DM=512).
#
# The network collapses algebraically: tokens_x[b] = diag(rw[b]) @ chart_emb is
# rank-structured, so every DM=512-wide projection folds into small per-chart
# constants computed once on the host:
#   scores[b,k] = rw[b,k] * (z[b] @ A + a0)[k] / sqrt(DM) - geo * acosh(arg)^2
#   arg[b,k]    = 1 + y,  y = 2*diff2[b,k] / ((1-|z[b]|^2) * (1-|c_k|^2))
#   out[b]      = sum_k softmax(scores)[b,k] * rw[b,k] * e[k] + e0
# with A [D,K], a0 [K], e [K], e0 scalar folded from the weight matrices
# (spectral norms included). Pure data parallel over B: each of the 8 cores
# processes 512 rows (4 tiles of 128 on partitions).
#
# Device program (v5):
#  - The izd = 2/(1-|z|^2) factor is folded into the matmul by scaling each
#    z-column (and the zn/ones rows) by izd on the host; the S1 columns pick
#    up the same factor, compensated by shipping rw' = rw/(sqrt(DM)*izd).
#    One fp16 66x128x128 matmul per tile, all four into ONE psum bank, so
#    y = psum[:, :, 64:128] is a single strided AP.
#  - geo*acosh(1+y)^2 is evaluated as a degree-6 polynomial (fit at build
#    time on the data's y-range bound, max err ~1e-5): 6 serial DVE ops via
#    h <- (h + c_j) * y. No sqrt/ln/square -> only EXP on ACT, one LUT set,
#    zero table switches on the critical path.
#  - S = sum_k p on gpsimd, N = sum_k p*rwe via 4 fused tensor_tensor_reduce
#    on DVE (runs in parallel with gpsimd).
#  - Result [128, 8] is PE-transposed to [8, 128] so the output DMA is 8
#    fat descriptors instead of 128 tiny ones.
#  - All DMAs on HWDGE (sync + scalar queues); no gpsimd SWDGE anywhere.
import json
import os
import sys
import tempfile

import numpy as np

for _p in ('/opt/trn_rl_repo', '/root/.axon_site/_ro/trn_rl_repo'):
    if _p not in sys.path:
        sys.path.append(_p)

import concourse.bass as bass
import concourse.mybir as mybir
import concourse.tile as tile
import concourse.bacc as bacc
from concourse.bass_utils import run_bass_kernel_spmd

F32 = mybir.dt.float32
F16 = mybir.dt.float16
I32 = mybir.dt.int32
N_CORES = 8
B, D, K, DM = 4096, 64, 64, 512
BC = B // N_CORES          # 512 rows per core
NT = BC // 128             # 4 tiles of 128 rows
ALU = mybir.AluOpType
ACTF = mybir.ActivationFunctionType
ACT_CFG_VERSION = 5        # bump when the act-table config changes (cache bust)
PDEG = 4                   # polynomial degree for geo*acosh(1+y)^2

ZZ_P = 66                  # zz partition rows: 64 z.T + zn + ones (all izd-scaled)
ZW = NT * 128 + 128        # zzg cols: 512 z-data + 128 coef block
RW_W = NT * K + K          # rwb cols: 256 rw' + 64 e broadcast


def _find_act_dir():
    import glob
    cands = glob.glob(
        '/nix/store/*/lib/python3*/site-packages/neuronxcc/pwp/pwp_bin_trainium')
    for c in cands:
        if os.path.exists(os.path.join(c, 'act_info.json')):
            return c
    return None


def _make_act_root():
    """Custom act_info.json with ONLY natural_log_exp_and_others: the kernel's
    sole ACT function is Exp, so a single LUT set means a single table load
    (warmed up during the input DMA). Returns (json_path, tables)."""
    src_dir = _find_act_dir()
    if src_dir is None:
        return None, None
    try:
        info = json.load(open(os.path.join(src_dir, 'act_info.json')))
        keep = [s for s in info['act_func_sets']
                if s.get('name') == 'natural_log_exp_and_others']
        if len(keep) != 1:
            return None, None
        out_dir = tempfile.mkdtemp(prefix='act_root_')
        for s in keep:
            for k in info['pwp_file_keys']:
                fn = s[k]
                os.symlink(os.path.join(src_dir, fn), os.path.join(out_dir, fn))
        json.dump({'pwp_file_keys': info['pwp_file_keys'], 'act_func_sets': keep},
                  open(os.path.join(out_dir, 'act_info.json'), 'w'))
        tables = [
            (s['name'], {ACTF.from_pwp(v) for v in s['act'].keys()})
            for s in keep
        ]
        return os.path.join(out_dir, 'act_info.json'), tables
    except Exception:
        return None, None


class _Bacc(bacc.Bacc):
    """Bacc whose activation-table placement uses the filtered act_info
    (ids must index the json walrus sees via BASS_ACT_ROOT_JSON_PATH)."""

    _act_tables = None

    def insert_act_table_loads(self):
        if self._act_tables is None:
            return super().insert_act_table_loads()
        import bass_rust as _bass_rust
        has_activation = any(
            isinstance(i, mybir.InstActivation)
            for b in self.main_func.blocks
            for i in b.instructions
        )
        if not has_activation:
            return
        _bass_rust.insert_act_table_loads(self, list(self._act_tables))


def _fold_constants(inputs):
    """Host-side folding of all weights into small per-chart constants, plus
    the polynomial fit for geo*acosh(1+y)^2 (float64 throughout)."""
    ii = {k: np.asarray(v).astype(np.float64) for k, v in inputs.items()}

    def l2n(x):
        return x / (np.linalg.norm(x) + 1e-12)

    def sscale(W, iters=5):
        u = l2n(np.ones(W.shape[0]))
        v = l2n(W.T @ u)
        for _ in range(iters):
            v = l2n(W.T @ u)
            u = l2n(W @ v)
        return W / (u @ (W @ v))

    Wz = sscale(ii['zW'])                     # [DM, D]
    vWs = sscale(ii['vW'])                    # [1, DM]
    cc = ii['chart_centers']
    n = np.linalg.norm(cc, axis=-1, keepdims=True)
    ccp = cc * np.minimum(1.0, (1.0 - 1e-5) / np.maximum(n, 1e-12))   # [K, D]
    cn = np.sum(ccp * ccp, axis=-1)           # [K]
    cdiv = 1.0 - cn                           # [K]

    Ek = ii['chart_emb'] @ ii['Wk'].T         # [K, DM]
    Ev = ii['chart_emb'] @ ii['Wv'].T         # [K, DM]
    A = Wz.T @ (ii['Wq'].T @ Ek.T)            # [D, K]
    a0 = (ii['zb'] @ ii['Wq'].T + ii['bq']) @ Ek.T     # [K]
    h = ii['Wo'].T @ vWs[0]                   # [DM]
    e = Ev @ h                                # [K]
    e0 = float(ii['bv'] @ h + ii['bo'] @ vWs[0] + ii['vb'][0])
    geo = float(ii['geo_scale'])

    # coef block [66, 128]: cols 0:64 -> S1 (z@A + a0), cols 64:128 -> y
    coef = np.zeros((ZZ_P, 128), dtype=np.float64)
    coef[0:D, 0:K] = A
    coef[D + 1, 0:K] = a0
    coef[0:D, K:128] = (-2.0 * ccp / cdiv[:, None]).T
    coef[D, K:128] = 1.0 / cdiv
    coef[D + 1, K:128] = cn / cdiv

    # y-range bound from per-row norms, then lstsq fit of geo*acosh(1+y)^2
    z = ii['z']
    zn = np.sum(z * z, axis=1)
    izd = 2.0 / np.maximum(1.0 - zn, 1e-6)
    ymax = float(np.max(2.0 * (np.sqrt(zn) + np.sqrt(cn.max())) ** 2
                        / (np.maximum(1.0 - zn, 1e-6) * cdiv.min()))) * 1.05
    g = np.linspace(0.0, max(ymax, 1e-3), 4001)
    tgt = geo * np.arccosh(np.maximum(1.0 + g, 1.0)) ** 2
    V = np.stack([g ** i for i in range(1, PDEG + 1)], 1)
    a, *_ = np.linalg.lstsq(V, tgt, rcond=None)
    # negated coefficients: device computes h = -geo*P(y), sco = h + sc
    cseq = [float(np.float32(-a[i])) for i in range(PDEG)]   # na[i] = -a_{i+1}

    return {
        'coef': coef, 'e': e, 'e0': e0, 'geo': geo,
        'zn': zn, 'izd': izd, 'cseq': cseq,
        'inv_sqrt': 1.0 / np.sqrt(float(DM)),
    }


def _pack_data(inputs, consts):
    """Per-core blocks: zzg [N,66,ZW] fp16 and rwb [N,128,RW_W] fp16."""
    z = np.asarray(inputs['z']).astype(np.float64)
    rw = np.asarray(inputs['rw']).astype(np.float64)
    zn, izd = consts['zn'], consts['izd']
    rwp = rw * (consts['inv_sqrt'] / izd[:, None])        # rw' compensation

    # kappa keeps rwe = rw'*(e*kappa) and pn = p*rwe inside fp16 normal range,
    # without overflowing the shipped e*kappa itself (fp16 max 65504)
    kappa = min(
        1024.0 / max(float(np.max(np.abs(rwp)) * np.max(np.abs(consts['e']))),
                     1e-30),
        49152.0 / max(float(np.max(np.abs(consts['e']))), 1e-30))
    consts['kappa'] = kappa

    zzg = np.zeros((N_CORES, ZZ_P, ZW), dtype=np.float16)
    rwb = np.zeros((N_CORES, 128, RW_W), dtype=np.float16)
    zi = (z * izd[:, None])                               # [B, D]
    for c in range(N_CORES):
        for t in range(NT):
            lo = c * BC + t * 128
            zzg[c, 0:D, t * 128:(t + 1) * 128] = zi[lo:lo + 128].T.astype(np.float16)
            zzg[c, D, t * 128:(t + 1) * 128] = (zn * izd)[lo:lo + 128].astype(np.float16)
            zzg[c, D + 1, t * 128:(t + 1) * 128] = izd[lo:lo + 128].astype(np.float16)
            rwb[c, :, t * K:(t + 1) * K] = rwp[lo:lo + 128].astype(np.float16)
        zzg[c, :, NT * 128:] = consts['coef'].astype(np.float16)
        rwb[c, :, NT * K:] = (consts['e'] * kappa).astype(np.float16)[None, :]
    return zzg, rwb


def _build_program(consts, act_tables=None):
    _Bacc._act_tables = act_tables
    nc = _Bacc()
    zzg_in = nc.dram_tensor("zzg_in", [ZZ_P, ZW], F16, kind="ExternalInput")
    rwb_in = nc.dram_tensor("rwb_in", [128, RW_W], F16, kind="ExternalInput")
    res_out = nc.dram_tensor("res_out", [2 * NT, 128], F32, kind="ExternalOutput")
    nc.inline_tensor(np.array([ACT_CFG_VERSION], dtype=np.int32), name="c_cfg")

    cseq = consts['cseq']

    with tile.TileContext(nc) as tc:
        with (
            tc.tile_pool(name="sb", bufs=1) as sb,
            tc.tile_pool(name="ps", bufs=2, space=bass.MemorySpace.PSUM) as ps,
        ):
            # Input DMAs first, one per HWDGE queue (sync + scalar) so the
            # descriptor generations run concurrently.
            zzg = sb.tile([ZZ_P, ZW], F16)
            nc.sync.dma_start(zzg[:], zzg_in[:])
            rwb = sb.tile([128, RW_W], F16)
            nc.scalar.dma_start(rwb[:], rwb_in[:])

            # ACT table warmup (single LUT set) while the DMAs stream.
            dummy = sb.tile([1, 1], F32)
            nc.gpsimd.memset(dummy[:], 0.0)
            nc.scalar.activation(dummy[:], dummy[:], ACTF.Exp)

            # Identity for the PE output transpose, built on idle gpsimd:
            # iota val = col - partition, is_equal 0 -> 1.0 on the diagonal.
            io = sb.tile([128, 128], I32)
            nc.gpsimd.iota(io[:], pattern=[[1, 128]], base=0, channel_multiplier=-1)
            ident = sb.tile([128, 128], F32)
            nc.gpsimd.tensor_scalar(out=ident[:], in0=io[:], scalar1=0,
                                    scalar2=None, op0=ALU.is_equal)

            rw_v = rwb[:, 0:NT * K].rearrange("p (t k) -> p t k", t=NT)
            e_b = rwb[:, NT * K:NT * K + K]
            coef = zzg[:, NT * 128:]

            # All four matmuls into ONE psum bank -> y is one strided AP.
            pt = ps.tile([128, NT, 128], F32)
            for t in range(NT):
                nc.tensor.matmul(pt[:, t, :], zzg[:, t * 128:(t + 1) * 128],
                                 coef, start=True, stop=True)
            y = pt[:, :, K:128]                   # [128, NT, 64] strided psum

            # DVE work that only needs rwb (overlaps the matmuls):
            rwe = sb.tile([128, NT, K], F16)
            e_bc = e_b.to_broadcast([128, K, NT]).rearrange("p k t -> p t k")
            nc.vector.tensor_tensor(out=rwe[:], in0=rw_v, in1=e_bc, op=ALU.mult)

            # sc = S1' * rw'  (izd folded through both factors)
            sc = sb.tile([128, NT, K], F16)
            nc.vector.tensor_tensor(out=sc[:], in0=pt[:, :, 0:K], in1=rw_v,
                                    op=ALU.mult)

            # fp16 copy of y: the 2x-rate DVE chain below reads it
            y16 = sb.tile([128, NT, K], F16)
            nc.vector.tensor_copy(y16[:], y)

            # -geo*acosh(1+y)^2 as deg-4 poly, even/odd split so every op is
            # TT or TS (2x-rate in fp16; scalar_tensor_tensor runs 1x):
            #   P = y*(na1 + na3*u) + u*(na2 + na4*u),  u = y^2
            na = cseq   # na[i] = -a_{i+1}
            u = sb.tile([128, NT, K], F16)
            nc.vector.tensor_tensor(out=u[:], in0=y16[:], in1=y16[:], op=ALU.mult)
            r1 = sb.tile([128, NT, K], F16)
            nc.vector.tensor_scalar(out=r1[:], in0=u[:], scalar1=na[2],
                                    scalar2=na[0], op0=ALU.mult, op1=ALU.add)
            r2 = sb.tile([128, NT, K], F16)
            nc.vector.tensor_scalar(out=r2[:], in0=u[:], scalar1=na[3],
                                    scalar2=na[1], op0=ALU.mult, op1=ALU.add)
            t1 = sb.tile([128, NT, K], F16)
            nc.vector.tensor_tensor(out=t1[:], in0=r1[:], in1=y16[:], op=ALU.mult)
            t2 = sb.tile([128, NT, K], F16)
            nc.vector.tensor_tensor(out=t2[:], in0=r2[:], in1=u[:], op=ALU.mult)
            h = sb.tile([128, NT, K], F16)
            nc.vector.tensor_tensor(out=h[:], in0=t1[:], in1=t2[:], op=ALU.add)
            nc.vector.tensor_tensor(out=h[:], in0=h[:], in1=sc[:], op=ALU.add)

            # softmax-weighted sum (scores in [-2.3,-0.4]: no max-shift needed)
            ex = sb.tile([128, NT, K], F16)
            nc.scalar.activation(ex[:], h[:], ACTF.Exp)
            sn = sb.tile([128, 2, NT], F32)       # cols 0:4 = S_t, 4:8 = N_t
            nc.vector.reduce_sum(sn[:, 0, :], ex[:], axis=mybir.AxisListType.X)
            pn = sb.tile([128, NT, K], F16)
            nc.vector.tensor_tensor(out=pn[:], in0=ex[:], in1=rwe[:], op=ALU.mult)
            nc.vector.reduce_sum(sn[:, 1, :], pn[:], axis=mybir.AxisListType.X)

            # transpose [128, 8] -> [8, 128]: output DMA becomes 8 descriptors
            ptr = ps.tile([2 * NT, 128], F32)
            nc.tensor.transpose(ptr[:], sn.rearrange("p a t -> p (a t)"), ident[:])
            res = sb.tile([2 * NT, 128], F32)
            nc.vector.tensor_copy(res[:], ptr[:])
            nc.sync.dma_start(res_out[:], res[:])

    nc.compile()
    return nc


def _run(inputs, trace=False):
    consts = _fold_constants(inputs)
    zzg, rwb = _pack_data(inputs, consts)
    act_root, act_tables = _make_act_root()
    saved = os.environ.get('BASS_ACT_ROOT_JSON_PATH')
    try:
        if act_root is not None:
            os.environ['BASS_ACT_ROOT_JSON_PATH'] = act_root
        nc = _build_program(consts, act_tables)
        in_maps = [{"zzg_in": np.ascontiguousarray(zzg[c]),
                    "rwb_in": np.ascontiguousarray(rwb[c])}
                   for c in range(N_CORES)]
        r = run_bass_kernel_spmd(nc, in_maps, core_ids=list(range(N_CORES)),
                                 trace=trace)
    finally:
        if saved is None:
            os.environ.pop('BASS_ACT_ROOT_JSON_PATH', None)
        else:
            os.environ['BASS_ACT_ROOT_JSON_PATH'] = saved
    out = np.empty((B, 1), dtype=np.float32)
    e0 = np.float32(consts['e0'])
    # rwe on device used rw' = rw*inv_sqrt/izd, so N is scaled by
    # inv_sqrt/izd_b per row — undo that here (host does the division anyway).
    unscale = (consts['izd'] / (consts['inv_sqrt'] * consts['kappa'])
               ).astype(np.float32)   # [B]
    for c in range(N_CORES):
        res = r.results[c]["res_out"]        # [8, 128]: rows 0:4 S_t, 4:8 N_t
        val = (res[NT:2 * NT, :] / res[0:NT, :]).astype(np.float32)   # [NT, 128]
        out[c * BC:(c + 1) * BC, 0] = (val.reshape(BC)
                                       * unscale[c * BC:(c + 1) * BC] + e0)
    return out, r


def kernel(**inputs):
    out, _ = _run(inputs, trace=False)
    return out


def run_traced(**inputs):
    return _run(inputs, trace=True)


# revision 15
# speedup vs baseline: 1.1598x; 1.1598x over previous
# Trainium2 Bass kernel for nn_CovariantPotentialNet (B=4096, D=64, K=64, DM=512).
#
# The network collapses algebraically: tokens_x[b] = diag(rw[b]) @ chart_emb is
# rank-structured, so every DM=512-wide projection folds into small per-chart
# constants computed once on the host:
#   scores[b,k] = rw[b,k] * (z[b] @ A + a0)[k] / sqrt(DM) - geo * acosh(arg)^2
#   arg[b,k]    = 1 + y,  y = 2*diff2[b,k] / ((1-|z[b]|^2) * (1-|c_k|^2))
#   out[b]      = sum_k softmax(scores)[b,k] * rw[b,k] * e[k] + e0
# with A [D,K], a0 [K], e [K], e0 scalar folded from the weight matrices
# (spectral norms included). Pure data parallel over B: each of the 8 cores
# processes 512 rows (4 tiles of 128 on partitions).
#
# Device program (v5):
#  - The izd = 2/(1-|z|^2) factor is folded into the matmul by scaling each
#    z-column (and the zn/ones rows) by izd on the host; the S1 columns pick
#    up the same factor, compensated by shipping rw' = rw/(sqrt(DM)*izd).
#    One fp16 66x128x128 matmul per tile, all four into ONE psum bank, so
#    y = psum[:, :, 64:128] is a single strided AP.
#  - geo*acosh(1+y)^2 is evaluated as a degree-6 polynomial (fit at build
#    time on the data's y-range bound, max err ~1e-5): 6 serial DVE ops via
#    h <- (h + c_j) * y. No sqrt/ln/square -> only EXP on ACT, one LUT set,
#    zero table switches on the critical path.
#  - S = sum_k p on gpsimd, N = sum_k p*rwe via 4 fused tensor_tensor_reduce
#    on DVE (runs in parallel with gpsimd).
#  - Result [128, 8] is PE-transposed to [8, 128] so the output DMA is 8
#    fat descriptors instead of 128 tiny ones.
#  - All DMAs on HWDGE (sync + scalar queues); no gpsimd SWDGE anywhere.
import json
import os
import sys
import tempfile

import numpy as np

for _p in ('/opt/trn_rl_repo', '/root/.axon_site/_ro/trn_rl_repo'):
    if _p not in sys.path:
        sys.path.append(_p)

import concourse.bass as bass
import concourse.mybir as mybir
import concourse.tile as tile
import concourse.bacc as bacc
from concourse.bass_utils import run_bass_kernel_spmd

F32 = mybir.dt.float32
F16 = mybir.dt.float16
I32 = mybir.dt.int32
N_CORES = 8
B, D, K, DM = 4096, 64, 64, 512
BC = B // N_CORES          # 512 rows per core
NT = BC // 128             # 4 tiles of 128 rows
ALU = mybir.AluOpType
ACTF = mybir.ActivationFunctionType
ACT_CFG_VERSION = 5        # bump when the act-table config changes (cache bust)
PDEG = 4                   # polynomial degree for geo*acosh(1+y)^2

ZZ_P = 66                  # zz partition rows: 64 z.T + zn + ones (all izd-scaled)
ZW = NT * 128 + 128        # zzg cols: 512 z-data + 128 coef block
RW_W = NT * K + K          # rwb cols: 256 rw' + 64 e broadcast


def _find_act_dir():
    import glob
    cands = glob.glob(
        '/nix/store/*/lib/python3*/site-packages/neuronxcc/pwp/pwp_bin_trainium')
    for c in cands:
        if os.path.exists(os.path.join(c, 'act_info.json')):
            return c
    return None


def _make_act_root():
    """Custom act_info.json with ONLY natural_log_exp_and_others: the kernel's
    sole ACT function is Exp, so a single LUT set means a single table load
    (warmed up during the input DMA). Returns (json_path, tables)."""
    src_dir = _find_act_dir()
    if src_dir is None:
        return None, None
    try:
        info = json.load(open(os.path.join(src_dir, 'act_info.json')))
        keep = [s for s in info['act_func_sets']
                if s.get('name') == 'natural_log_exp_and_others']
        if len(keep) != 1:
            return None, None
        out_dir = tempfile.mkdtemp(prefix='act_root_')
        for s in keep:
            for k in info['pwp_file_keys']:
                fn = s[k]
                os.symlink(os.path.join(src_dir, fn), os.path.join(out_dir, fn))
        json.dump({'pwp_file_keys': info['pwp_file_keys'], 'act_func_sets': keep},
                  open(os.path.join(out_dir, 'act_info.json'), 'w'))
        tables = [
            (s['name'], {ACTF.from_pwp(v) for v in s['act'].keys()})
            for s in keep
        ]
        return os.path.join(out_dir, 'act_info.json'), tables
    except Exception:
        return None, None


class _Bacc(bacc.Bacc):
    """Bacc whose activation-table placement uses the filtered act_info
    (ids must index the json walrus sees via BASS_ACT_ROOT_JSON_PATH)."""

    _act_tables = None

    def insert_act_table_loads(self):
        if self._act_tables is None:
            return super().insert_act_table_loads()
        import bass_rust as _bass_rust
        has_activation = any(
            isinstance(i, mybir.InstActivation)
            for b in self.main_func.blocks
            for i in b.instructions
        )
        if not has_activation:
            return
        _bass_rust.insert_act_table_loads(self, list(self._act_tables))


def _fold_constants(inputs):
    """Host-side folding of all weights into small per-chart constants, plus
    the polynomial fit for geo*acosh(1+y)^2 (float64 throughout)."""
    ii = {k: np.asarray(v).astype(np.float64) for k, v in inputs.items()}

    def l2n(x):
        return x / (np.linalg.norm(x) + 1e-12)

    def sscale(W, iters=5):
        u = l2n(np.ones(W.shape[0]))
        v = l2n(W.T @ u)
        for _ in range(iters):
            v = l2n(W.T @ u)
            u = l2n(W @ v)
        return W / (u @ (W @ v))

    Wz = sscale(ii['zW'])                     # [DM, D]
    vWs = sscale(ii['vW'])                    # [1, DM]
    cc = ii['chart_centers']
    n = np.linalg.norm(cc, axis=-1, keepdims=True)
    ccp = cc * np.minimum(1.0, (1.0 - 1e-5) / np.maximum(n, 1e-12))   # [K, D]
    cn = np.sum(ccp * ccp, axis=-1)           # [K]
    cdiv = 1.0 - cn                           # [K]

    Ek = ii['chart_emb'] @ ii['Wk'].T         # [K, DM]
    Ev = ii['chart_emb'] @ ii['Wv'].T         # [K, DM]
    A = Wz.T @ (ii['Wq'].T @ Ek.T)            # [D, K]
    a0 = (ii['zb'] @ ii['Wq'].T + ii['bq']) @ Ek.T     # [K]
    h = ii['Wo'].T @ vWs[0]                   # [DM]
    e = Ev @ h                                # [K]
    e0 = float(ii['bv'] @ h + ii['bo'] @ vWs[0] + ii['vb'][0])
    geo = float(ii['geo_scale'])

    # coef block [66, 128]: cols 0:64 -> S1 (z@A + a0), cols 64:128 -> y
    coef = np.zeros((ZZ_P, 128), dtype=np.float64)
    coef[0:D, 0:K] = A
    coef[D + 1, 0:K] = a0
    coef[0:D, K:128] = (-2.0 * ccp / cdiv[:, None]).T
    coef[D, K:128] = 1.0 / cdiv
    coef[D + 1, K:128] = cn / cdiv

    # y-range bound from per-row norms, then lstsq fit of geo*acosh(1+y)^2
    z = ii['z']
    zn = np.sum(z * z, axis=1)
    izd = 2.0 / np.maximum(1.0 - zn, 1e-6)
    ymax = float(np.max(2.0 * (np.sqrt(zn) + np.sqrt(cn.max())) ** 2
                        / (np.maximum(1.0 - zn, 1e-6) * cdiv.min()))) * 1.05
    g = np.linspace(0.0, max(ymax, 1e-3), 4001)
    tgt = geo * np.arccosh(np.maximum(1.0 + g, 1.0)) ** 2
    V = np.stack([g ** i for i in range(1, PDEG + 1)], 1)
    a, *_ = np.linalg.lstsq(V, tgt, rcond=None)
    # negated coefficients: device computes h = -geo*P(y), sco = h + sc
    cseq = [float(np.float32(-a[i])) for i in range(PDEG)]   # na[i] = -a_{i+1}

    return {
        'coef': coef, 'e': e, 'e0': e0, 'geo': geo,
        'zn': zn, 'izd': izd, 'cseq': cseq,
        'inv_sqrt': 1.0 / np.sqrt(float(DM)),
    }


def _pack_data(inputs, consts):
    """Per-core blocks: zzg [N,66,ZW] fp16 and rwb [N,128,RW_W] fp16."""
    z = np.asarray(inputs['z']).astype(np.float64)
    rw = np.asarray(inputs['rw']).astype(np.float64)
    zn, izd = consts['zn'], consts['izd']
    rwp = rw * (consts['inv_sqrt'] / izd[:, None])        # rw' compensation

    # kappa keeps rwe = rw'*(e*kappa) and pn = p*rwe inside fp16 normal range,
    # without overflowing the shipped e*kappa itself (fp16 max 65504)
    kappa = min(
        1024.0 / max(float(np.max(np.abs(rwp)) * np.max(np.abs(consts['e']))),
                     1e-30),
        49152.0 / max(float(np.max(np.abs(consts['e']))), 1e-30))
    consts['kappa'] = kappa

    zzg = np.zeros((N_CORES, ZZ_P, ZW), dtype=np.float16)
    rwb = np.zeros((N_CORES, 128, RW_W), dtype=np.float16)
    zi = (z * izd[:, None])                               # [B, D]
    for c in range(N_CORES):
        for t in range(NT):
            lo = c * BC + t * 128
            zzg[c, 0:D, t * 128:(t + 1) * 128] = zi[lo:lo + 128].T.astype(np.float16)
            zzg[c, D, t * 128:(t + 1) * 128] = (zn * izd)[lo:lo + 128].astype(np.float16)
            zzg[c, D + 1, t * 128:(t + 1) * 128] = izd[lo:lo + 128].astype(np.float16)
            rwb[c, :, t * K:(t + 1) * K] = rwp[lo:lo + 128].astype(np.float16)
        zzg[c, :, NT * 128:] = consts['coef'].astype(np.float16)
        rwb[c, :, NT * K:] = (consts['e'] * kappa).astype(np.float16)[None, :]
    return zzg, rwb


def _build_program(consts, act_tables=None):
    """Raw bass (no TileContext): manual semaphores avoid ~1us of tile
    preamble/epilogue. Engine streams are in-order; sems only cross engines."""
    _Bacc._act_tables = act_tables
    nc = _Bacc()
    zzg_in = nc.dram_tensor("zzg_in", [ZZ_P, ZW], F16, kind="ExternalInput")
    rwb_in = nc.dram_tensor("rwb_in", [128, RW_W], F16, kind="ExternalInput")
    res_out = nc.dram_tensor("res_out", [128, 2, NT], F32, kind="ExternalOutput")
    nc.inline_tensor(np.array([ACT_CFG_VERSION], dtype=np.int32), name="c_cfg")
    na = consts['cseq']

    zzg = nc.alloc_sbuf_tensor("zzg", [ZZ_P, ZW], F16)
    rwb = nc.alloc_sbuf_tensor("rwb", [128, RW_W], F16)
    warm = nc.alloc_sbuf_tensor("warm_sb", [128, 1], F32)
    y16 = nc.alloc_sbuf_tensor("y16", [128, NT, K], F16)
    u_t = nc.alloc_sbuf_tensor("u_t", [128, NT, K], F16)
    r1t = nc.alloc_sbuf_tensor("r1t", [128, NT, K], F16)
    r2t = nc.alloc_sbuf_tensor("r2t", [128, NT, K], F16)
    t1t = nc.alloc_sbuf_tensor("t1t", [128, NT, K], F16)
    t2t = nc.alloc_sbuf_tensor("t2t", [128, NT, K], F16)
    h_t = nc.alloc_sbuf_tensor("h_t", [128, NT, K], F16)
    sc_t = nc.alloc_sbuf_tensor("sc_t", [128, NT, K], F16)
    ex_t = nc.alloc_sbuf_tensor("ex_t", [128, NT, K], F16)
    rwe = nc.alloc_sbuf_tensor("rwe", [128, NT, K], F16)
    pn_t = nc.alloc_sbuf_tensor("pn_t", [128, NT, K], F16)
    sn = nc.alloc_sbuf_tensor("sn", [128, 2, NT], F32)
    pts = [nc.alloc_psum_tensor(f"pt{t}", [128, 128], F32) for t in range(NT)]

    zza_sem = nc.alloc_semaphore("zza_sem")
    zzb_sem = nc.alloc_semaphore("zzb_sem")
    rwb_sem = nc.alloc_semaphore("rwb_sem")
    mm_sem = nc.alloc_semaphore("mm_sem")
    h_sem = nc.alloc_semaphore("h_sem")
    ex_sem = nc.alloc_semaphore("ex_sem")
    sn_sem = nc.alloc_semaphore("sn_sem")
    out_sem = nc.alloc_semaphore("out_sem")

    rw_v = rwb.ap()[:, 0:NT * K].rearrange("p (t k) -> p t k", t=NT)
    e_b = rwb.ap()[:, NT * K:NT * K + K]
    coef = zzg.ap()[:, NT * 128:]
    ZP = ZZ_P // 2          # partition split for the two zzg DMA halves

    with nc.Block() as blk:
        @blk.sync
        def _(sync):
            sync.dma_start(zzg.ap()[0:ZP, :],
                           zzg_in.ap()[0:ZP, :]).then_inc(zza_sem, 16)
            sync.dma_start(rwb.ap(), rwb_in.ap()).then_inc(rwb_sem, 16)
            sync.wait_ge(sn_sem, 1)
            sync.dma_start(res_out.ap(), sn.ap()).then_inc(out_sem, 16)
            sync.wait_ge(out_sem, 16)

        @blk.scalar
        def _(scalar):
            scalar.dma_start(zzg.ap()[ZP:, :],
                             zzg_in.ap()[ZP:, :]).then_inc(zzb_sem, 16)
            # warmup: triggers the single ACT LUT load during the input DMA
            scalar.activation(warm.ap(), nc.const_aps.aps[(F32, 0.0)],
                              ACTF.Exp)
            scalar.wait_ge(h_sem, 1)
            scalar.activation(ex_t.ap(), h_t.ap(), ACTF.Exp).then_inc(ex_sem, 1)

        @blk.tensor
        def _(tensor):
            tensor.wait_ge(zza_sem, 16)
            tensor.wait_ge(zzb_sem, 16)
            for t in range(NT):
                tensor.matmul(pts[t].ap(),
                              zzg.ap()[:, t * 128:(t + 1) * 128],
                              coef, start=True,
                              stop=True).then_inc(mm_sem, 1)

        @blk.vector
        def _(vector):
            # per-tile psum reads overlap the matmul pipeline
            for t in range(NT):
                vector.wait_ge(mm_sem, t + 1)
                vector.tensor_copy(y16.ap()[:, t, :], pts[t].ap()[:, K:128])
                vector.tensor_tensor(out=sc_t.ap()[:, t, :],
                                     in0=pts[t].ap()[:, 0:K],
                                     in1=rw_v[:, t, :], op=ALU.mult)
            #   P = y*(na1 + na3*u) + u*(na2 + na4*u),  u = y^2  (all TT/TS)
            vector.tensor_tensor(out=u_t.ap(), in0=y16.ap(), in1=y16.ap(),
                                 op=ALU.mult)
            vector.tensor_scalar(out=r1t.ap(), in0=u_t.ap(), scalar1=na[2],
                                 scalar2=na[0], op0=ALU.mult, op1=ALU.add)
            vector.tensor_scalar(out=r2t.ap(), in0=u_t.ap(), scalar1=na[3],
                                 scalar2=na[1], op0=ALU.mult, op1=ALU.add)
            vector.tensor_tensor(out=t1t.ap(), in0=r1t.ap(), in1=y16.ap(),
                                 op=ALU.mult)
            vector.tensor_tensor(out=t2t.ap(), in0=r2t.ap(), in1=u_t.ap(),
                                 op=ALU.mult)
            vector.tensor_tensor(out=h_t.ap(), in0=t1t.ap(), in1=t2t.ap(),
                                 op=ALU.add)
            vector.tensor_tensor(out=h_t.ap(), in0=h_t.ap(), in1=sc_t.ap(),
                                 op=ALU.add).then_inc(h_sem, 1)
            vector.wait_ge(rwb_sem, 16)
            e_bc = e_b.to_broadcast([128, K, NT]).rearrange("p k t -> p t k")
            vector.tensor_tensor(out=rwe.ap(), in0=rw_v, in1=e_bc, op=ALU.mult)
            vector.wait_ge(ex_sem, 1)
            vector.tensor_tensor(out=pn_t.ap(), in0=ex_t.ap(), in1=rwe.ap(),
                                 op=ALU.mult)
            vector.reduce_sum(sn.ap()[:, 1, :], pn_t.ap(),
                              axis=mybir.AxisListType.X)
            vector.reduce_sum(sn.ap()[:, 0, :], ex_t.ap(),
                              axis=mybir.AxisListType.X).then_inc(sn_sem, 1)

    nc.compile()
    return nc


def _run(inputs, trace=False):
    consts = _fold_constants(inputs)
    zzg, rwb = _pack_data(inputs, consts)
    act_root, act_tables = _make_act_root()
    saved = os.environ.get('BASS_ACT_ROOT_JSON_PATH')
    try:
        if act_root is not None:
            os.environ['BASS_ACT_ROOT_JSON_PATH'] = act_root
        nc = _build_program(consts, act_tables)
        in_maps = [{"zzg_in": np.ascontiguousarray(zzg[c]),
                    "rwb_in": np.ascontiguousarray(rwb[c])}
                   for c in range(N_CORES)]
        r = run_bass_kernel_spmd(nc, in_maps, core_ids=list(range(N_CORES)),
                                 trace=trace)
    finally:
        if saved is None:
            os.environ.pop('BASS_ACT_ROOT_JSON_PATH', None)
        else:
            os.environ['BASS_ACT_ROOT_JSON_PATH'] = saved
    out = np.empty((B, 1), dtype=np.float32)
    e0 = np.float32(consts['e0'])
    # rwe on device used rw' = rw*inv_sqrt/izd, so N is scaled by
    # inv_sqrt/izd_b per row — undo that here (host does the division anyway).
    unscale = (consts['izd'] / (consts['inv_sqrt'] * consts['kappa'])
               ).astype(np.float32)   # [B]
    for c in range(N_CORES):
        res = r.results[c]["res_out"]        # [128, 2, NT]: S at [:,0,:], N [:,1,:]
        val = (res[:, 1, :] / res[:, 0, :]).astype(np.float32)        # [128, NT]
        out[c * BC:(c + 1) * BC, 0] = (val.T.reshape(BC)
                                       * unscale[c * BC:(c + 1) * BC] + e0)
    return out, r


def kernel(**inputs):
    out, _ = _run(inputs, trace=False)
    return out


def run_traced(**inputs):
    return _run(inputs, trace=True)
